# revision 4
# baseline (speedup 1.0000x reference)
"""EMA Vector-Quantizer forward kernel for 8 Trainium2 NeuronCores.

Data-parallel: z [32768, 512] is sharded along N across 8 cores (4096 rows
each); the codebook [8192, 512] is replicated. Each core computes its
distance block via fp32 PE matmuls, evicts PSUM through the scalar engine
with a fused (p - ||z||^2) bias, takes the row argmin with DVE max8 +
max_index (first-index tie semantics, matching jnp.argmin), gathers the
selected codebook rows with an indirect DMA, and applies the
straight-through estimator. Commit-loss partial sums are reduced on host.

Numerical notes (these choices reproduce the fp32 reference bit-for-bit
almost everywhere):
  * reference d = (||z||^2 + ||c||^2) - 2*z@c.T.  ||c||^2 <= 2.9e-6 while
    ||z||^2 >= ~385, so fl(zz + cc) == zz for every element; d == fl(zz - p)
    with p = 2*fl(z@c.T).
  * we feed the PE (2*z)^T so PSUM accumulates exactly 2*fl(z@c.T) (scaling
    by a power of two commutes with fp32 rounding).
  * s = fl(p - zz) = -d bitwise (activation Identity with bias=-zz is a
    single-rounding FMA), so first-argmax of s == first-argmin of d.
  * zz is a strict sequential fp32 sum (DVE tensor_reduce is sequential on
    HW), bit-identical to the device reference's jnp.sum.
"""

import numpy as np

N, K, D = 32768, 8192, 512
NCORES = 8
NLOC = N // NCORES          # 4096 rows per core
P = 128                     # partitions / row-tile height
RT = NLOC // P              # 32 row-tiles per core
KC = 512                    # codebook chunk (one fp32 PSUM bank)
NKC = K // KC               # 16 chunks
DC = D // P                 # 4 contraction chunks

_CACHE = {}


def _build():
    import concourse.bacc as bacc
    import concourse.bass as bass
    import concourse.mybir as mybir
    import concourse.tile as tile

    nc = bacc.Bacc("TRN2", target_bir_lowering=False)

    z2t_d = nc.dram_tensor("z2t", (RT, P, DC, P), mybir.dt.float32, kind="ExternalInput")
    zt_d = nc.dram_tensor("zt", (NLOC, D), mybir.dt.float32, kind="ExternalInput")
    ct_d = nc.dram_tensor("ct", (P, DC, K), mybir.dt.float32, kind="ExternalInput")
    cb_d = nc.dram_tensor("cb", (K, D), mybir.dt.float32, kind="ExternalInput")

    zst_o = nc.dram_tensor("zst", (NLOC, D), mybir.dt.float32, kind="ExternalOutput")
    idx_o = nc.dram_tensor("idx", (NLOC, 1), mybir.dt.uint32, kind="ExternalOutput")
    lp_o = nc.dram_tensor("lp", (P, RT), mybir.dt.float32, kind="ExternalOutput")

    with tile.TileContext(nc) as tc:
        with tc.tile_pool(name="ct", bufs=1) as ct_pool, \
             tc.tile_pool(name="sfull", bufs=1) as s_pool, \
             tc.tile_pool(name="work", bufs=2) as work, \
             tc.tile_pool(name="small", bufs=1) as small, \
             tc.tile_pool(name="ps", bufs=6, space="PSUM") as ps:

            ct_t = ct_pool.tile([P, DC, K], mybir.dt.float32, tag="ct")
            for j in range(8):
                w = K // 8
                nc.sync.dma_start(ct_t[:, :, j * w:(j + 1) * w],
                                  ct_d[:, :, j * w:(j + 1) * w])

            s_full = s_pool.tile([P, K], mybir.dt.float32, tag="sfull")
            losspart = small.tile([P, RT], mybir.dt.float32, tag="lp")

            for r in range(RT):
                z2t_t = work.tile([P, DC, P], mybir.dt.float32, tag="z2t")
                nc.sync.dma_start(z2t_t[:], z2t_d[r])
                zt_t = work.tile([P, D], mybir.dt.float32, tag="zt")
                nc.sync.dma_start(zt_t[:], zt_d[r * P:(r + 1) * P, :])

                zsq_t = work.tile([P, D], mybir.dt.float32, tag="zsq")
                nc.vector.tensor_tensor(out=zsq_t[:], in0=zt_t[:], in1=zt_t[:],
                                        op=mybir.AluOpType.mult)
                negzz_t = work.tile([P, 1], mybir.dt.float32, tag="negzz")
                nc.vector.tensor_reduce(out=negzz_t[:], in_=zsq_t[:],
                                        axis=mybir.AxisListType.X,
                                        op=mybir.AluOpType.add, negate=True)

                for kc in range(NKC):
                    p_t = ps.tile([P, KC], mybir.dt.float32, tag="p")
                    for d in range(DC):
                        nc.tensor.matmul(p_t[:], lhsT=z2t_t[:, d, :],
                                         rhs=ct_t[:, d, kc * KC:(kc + 1) * KC],
                                         start=(d == 0), stop=(d == DC - 1))
                    nc.scalar.activation(s_full[:, kc * KC:(kc + 1) * KC], p_t[:],
                                         mybir.ActivationFunctionType.Identity,
                                         bias=negzz_t[:], scale=1.0)

                mx_t = work.tile([P, 8], mybir.dt.float32, tag="mx")
                nc.vector.max(out=mx_t[:], in_=s_full[:])
                mi_t = work.tile([P, 8], mybir.dt.uint32, tag="mi")
                nc.vector.max_index(out=mi_t[:], in_max=mx_t[:], in_values=s_full[:])

                zq_t = work.tile([P, D], mybir.dt.float32, tag="zq")
                nc.gpsimd.indirect_dma_start(
                    out=zq_t[:], out_offset=None, in_=cb_d[:],
                    in_offset=bass.IndirectOffsetOnAxis(ap=mi_t[:, 0:1], axis=0))

                t_t = work.tile([P, D], mybir.dt.float32, tag="t")
                nc.vector.tensor_tensor(out=t_t[:], in0=zq_t[:], in1=zt_t[:],
                                        op=mybir.AluOpType.subtract)
                zst_t = work.tile([P, D], mybir.dt.float32, tag="zst")
                nc.vector.tensor_tensor(out=zst_t[:], in0=zt_t[:], in1=t_t[:],
                                        op=mybir.AluOpType.add)
                sq_t = work.tile([P, D], mybir.dt.float32, tag="sq")
                nc.scalar.activation(sq_t[:], t_t[:],
                                     mybir.ActivationFunctionType.Square,
                                     accum_out=losspart[:, r:r + 1])

                nc.sync.dma_start(zst_o[r * P:(r + 1) * P, :], zst_t[:])
                nc.sync.dma_start(idx_o[r * P:(r + 1) * P, :], mi_t[:, 0:1])

            nc.sync.dma_start(lp_o[:], losspart[:])

    nc.compile()
    return nc


def _get_nc():
    if "nc" not in _CACHE:
        _CACHE["nc"] = _build()
    return _CACHE["nc"]


def _prep_inputs(z, codebook):
    z = np.ascontiguousarray(z, dtype=np.float32)
    codebook = np.ascontiguousarray(codebook, dtype=np.float32)
    # [P, DC, K] codebook-transpose laid out exactly like the SBUF tile
    ct = np.ascontiguousarray(codebook.T.reshape(DC, P, K).transpose(1, 0, 2))
    in_maps = []
    for c in range(NCORES):
        zs = z[c * NLOC:(c + 1) * NLOC]                      # [4096, 512]
        z2 = 2.0 * zs                                         # exact in fp32
        # [RT, P, DC, P]: row-tile r, partition=d within chunk, chunk, row
        z2t = np.ascontiguousarray(
            z2.T.reshape(DC, P, RT, P).transpose(2, 1, 0, 3))
        in_maps.append({"z2t": z2t, "zt": np.ascontiguousarray(zs),
                        "ct": ct, "cb": codebook})
    return in_maps


def kernel(z, codebook):
    from concourse import bass_utils

    nc = _get_nc()
    in_maps = _prep_inputs(z, codebook)
    res = bass_utils.run_bass_kernel_spmd(nc, in_maps, core_ids=list(range(NCORES)))
    outs = res.results

    z_st = np.concatenate([o["zst"] for o in outs], axis=0)
    indices = np.concatenate([o["idx"][:, 0] for o in outs]).view(np.int32)
    total = np.float64(0.0)
    for o in outs:
        total += o["lp"].astype(np.float64).sum()
    commit_loss = np.float32(0.25 * total / (N * D))
    return z_st, indices.astype(np.int32), commit_loss


# revision 6
# speedup vs baseline: 1.2677x; 1.2677x over previous
"""EMA Vector-Quantizer forward kernel for 8 Trainium2 NeuronCores.

Data-parallel: z [32768, 512] is sharded along N across 8 cores (4096 rows
each); the codebook [8192, 512] is replicated. Each core computes its
distance block via fp32 PE matmuls, evicts PSUM through the scalar engine
with a fused (p - ||z||^2) bias, takes the row argmin with DVE max8 +
max_index (first-index tie semantics, matching jnp.argmin), gathers the
selected codebook rows with an indirect DMA, and applies the
straight-through estimator. Commit-loss partial sums are reduced on host.

Numerical notes (these choices reproduce the fp32 reference bit-for-bit
almost everywhere):
  * reference d = (||z||^2 + ||c||^2) - 2*z@c.T.  ||c||^2 <= 2.9e-6 while
    ||z||^2 >= ~385, so fl(zz + cc) == zz for every element; d == fl(zz - p)
    with p = 2*fl(z@c.T).
  * we feed the PE (2*z)^T so PSUM accumulates exactly 2*fl(z@c.T) (scaling
    by a power of two commutes with fp32 rounding).
  * s = fl(p - zz) = -d bitwise (activation Identity with bias=-zz is a
    single-rounding FMA), so first-argmax of s == first-argmin of d.
  * zz is a strict sequential fp32 sum (DVE tensor_reduce is sequential on
    HW), bit-identical to the device reference's jnp.sum.
"""

import numpy as np

N, K, D = 32768, 8192, 512
NCORES = 8
NLOC = N // NCORES          # 4096 rows per core
P = 128                     # partitions / row-tile height
RT = NLOC // P              # 32 row-tiles per core
KC = 512                    # codebook chunk (one fp32 PSUM bank)
NKC = K // KC               # 16 chunks
DC = D // P                 # 4 contraction chunks

_CACHE = {}


def _build():
    import concourse.bacc as bacc
    import concourse.bass as bass
    import concourse.mybir as mybir
    import concourse.tile as tile

    nc = bacc.Bacc("TRN2", target_bir_lowering=False)

    z2t_d = nc.dram_tensor("z2t", (RT, P, DC, P), mybir.dt.float32, kind="ExternalInput")
    zt_d = nc.dram_tensor("zt", (NLOC, D), mybir.dt.float32, kind="ExternalInput")
    ct_d = nc.dram_tensor("ct", (P, DC, K), mybir.dt.float32, kind="ExternalInput")
    cb_d = nc.dram_tensor("cb", (K, D), mybir.dt.float32, kind="ExternalInput")

    zst_o = nc.dram_tensor("zst", (NLOC, D), mybir.dt.float32, kind="ExternalOutput")
    idx_o = nc.dram_tensor("idx", (NLOC, 1), mybir.dt.uint32, kind="ExternalOutput")
    lp_o = nc.dram_tensor("lp", (P, RT), mybir.dt.float32, kind="ExternalOutput")

    with tile.TileContext(nc) as tc:
        with tc.tile_pool(name="ct", bufs=1) as ct_pool, \
             tc.tile_pool(name="sfull", bufs=1) as s_pool, \
             tc.tile_pool(name="work", bufs=2) as work, \
             tc.tile_pool(name="small", bufs=1) as small, \
             tc.tile_pool(name="ps", bufs=1, space="PSUM") as ps:

            ct_t = ct_pool.tile([P, DC, K], mybir.dt.float32, tag="ct")
            for j in range(8):
                w = K // 8
                nc.sync.dma_start(ct_t[:, :, j * w:(j + 1) * w],
                                  ct_d[:, :, j * w:(j + 1) * w])

            s_full = s_pool.tile([P, K], mybir.dt.float32, tag="sfull")
            losspart = small.tile([P, RT], mybir.dt.float32, tag="lp")

            for r in range(RT):
                z2t_t = work.tile([P, DC, P], mybir.dt.float32, tag="z2t")
                nc.sync.dma_start(z2t_t[:], z2t_d[r])
                zt_t = work.tile([P, D], mybir.dt.float32, tag="zt")
                nc.sync.dma_start(zt_t[:], zt_d[r * P:(r + 1) * P, :])

                zsq_t = work.tile([P, D], mybir.dt.float32, tag="zsq")
                nc.vector.tensor_tensor(out=zsq_t[:], in0=zt_t[:], in1=zt_t[:],
                                        op=mybir.AluOpType.mult)
                negzz_t = work.tile([P, 1], mybir.dt.float32, tag="negzz")
                nc.vector.tensor_reduce(out=negzz_t[:], in_=zsq_t[:],
                                        axis=mybir.AxisListType.X,
                                        op=mybir.AluOpType.add, negate=True)

                # d-outer, 8 PSUM banks: consecutive matmuls hit different
                # banks so fill/drain overlap (same-bank accumulate chains
                # serialize the PE).
                for g in range(NKC // 8):
                    p_ts = [ps.tile([P, KC], mybir.dt.float32, tag=f"p{i}",
                                    name=f"p_{r}_{g}_{i}") for i in range(8)]
                    for d in range(DC):
                        for i in range(8):
                            kc = g * 8 + i
                            nc.tensor.matmul(p_ts[i][:], lhsT=z2t_t[:, d, :],
                                             rhs=ct_t[:, d, kc * KC:(kc + 1) * KC],
                                             start=(d == 0), stop=(d == DC - 1))
                    for i in range(8):
                        kc = g * 8 + i
                        nc.scalar.activation(s_full[:, kc * KC:(kc + 1) * KC],
                                             p_ts[i][:],
                                             mybir.ActivationFunctionType.Identity,
                                             bias=negzz_t[:], scale=1.0)

                mx_t = work.tile([P, 8], mybir.dt.float32, tag="mx")
                nc.vector.max(out=mx_t[:], in_=s_full[:])
                mi_t = work.tile([P, 8], mybir.dt.uint32, tag="mi")
                nc.vector.max_index(out=mi_t[:], in_max=mx_t[:], in_values=s_full[:])

                zq_t = work.tile([P, D], mybir.dt.float32, tag="zq")
                nc.gpsimd.indirect_dma_start(
                    out=zq_t[:], out_offset=None, in_=cb_d[:],
                    in_offset=bass.IndirectOffsetOnAxis(ap=mi_t[:, 0:1], axis=0))

                t_t = work.tile([P, D], mybir.dt.float32, tag="t")
                nc.vector.tensor_tensor(out=t_t[:], in0=zq_t[:], in1=zt_t[:],
                                        op=mybir.AluOpType.subtract)
                zst_t = work.tile([P, D], mybir.dt.float32, tag="zst")
                nc.vector.tensor_tensor(out=zst_t[:], in0=zt_t[:], in1=t_t[:],
                                        op=mybir.AluOpType.add)
                sq_t = work.tile([P, D], mybir.dt.float32, tag="sq")
                nc.scalar.activation(sq_t[:], t_t[:],
                                     mybir.ActivationFunctionType.Square,
                                     accum_out=losspart[:, r:r + 1])

                nc.sync.dma_start(zst_o[r * P:(r + 1) * P, :], zst_t[:])
                nc.sync.dma_start(idx_o[r * P:(r + 1) * P, :], mi_t[:, 0:1])

            nc.sync.dma_start(lp_o[:], losspart[:])

    nc.compile()
    return nc


def _get_nc():
    if "nc" not in _CACHE:
        _CACHE["nc"] = _build()
    return _CACHE["nc"]


def _prep_inputs(z, codebook):
    z = np.ascontiguousarray(z, dtype=np.float32)
    codebook = np.ascontiguousarray(codebook, dtype=np.float32)
    # [P, DC, K] codebook-transpose laid out exactly like the SBUF tile
    ct = np.ascontiguousarray(codebook.T.reshape(DC, P, K).transpose(1, 0, 2))
    in_maps = []
    for c in range(NCORES):
        zs = z[c * NLOC:(c + 1) * NLOC]                      # [4096, 512]
        z2 = 2.0 * zs                                         # exact in fp32
        # [RT, P, DC, P]: row-tile r, partition=d within chunk, chunk, row
        z2t = np.ascontiguousarray(
            z2.T.reshape(DC, P, RT, P).transpose(2, 1, 0, 3))
        in_maps.append({"z2t": z2t, "zt": np.ascontiguousarray(zs),
                        "ct": ct, "cb": codebook})
    return in_maps


def kernel(z, codebook):
    from concourse import bass_utils

    nc = _get_nc()
    in_maps = _prep_inputs(z, codebook)
    res = bass_utils.run_bass_kernel_spmd(nc, in_maps, core_ids=list(range(NCORES)))
    outs = res.results

    z_st = np.concatenate([o["zst"] for o in outs], axis=0)
    indices = np.concatenate([o["idx"][:, 0] for o in outs]).view(np.int32)
    total = np.float64(0.0)
    for o in outs:
        total += o["lp"].astype(np.float64).sum()
    commit_loss = np.float32(0.25 * total / (N * D))
    return z_st, indices.astype(np.int32), commit_loss
